# revision 1
# baseline (speedup 1.0000x reference)
import numpy as np

N = 100000
D = 64
NG = 64
NC = 8
NPC = N // NC          # 12500 real nodes per core
NB = 98                # blocks of 128 dst nodes per core
NPAD = NB * 128        # 12544 padded nodes per core
NCH = 4                # src chunks (2 cores each)
CHROWS = 2 * NPAD      # 25088 table rows per chunk (< int16 max)
SLAB_BLOCKS = 7        # dst blocks per gather slab
RG = [[0, 1, 2, 3, 4, 5, 6, 7]]


def _preprocess(edge_index):
    src = np.concatenate([edge_index[0].astype(np.int64), np.arange(N, dtype=np.int64)])
    dst = np.concatenate([edge_index[1].astype(np.int64), np.arange(N, dtype=np.int64)])
    deg = np.bincount(dst, minlength=N)
    dis = (1.0 / np.sqrt(deg.astype(np.float64))).astype(np.float32)

    core_of = np.arange(N) // NPC
    bb_of = np.empty(N, np.int64)
    p_of = np.empty(N, np.int64)
    for c in range(NC):
        nodes = np.arange(c * NPC, (c + 1) * NPC)
        order = np.argsort(-deg[nodes], kind="stable")
        r = np.empty(NPC, np.int64)
        r[order] = np.arange(NPC)
        bb_of[nodes] = r % NB
        p_of[nodes] = r // NB
    row_of = core_of * NPAD + p_of * NB + bb_of

    edata = []
    maxcnt = 0
    dst_core = core_of[dst]
    for c in range(NC):
        m = dst_core == c
        es, ed = src[m], dst[m]
        ch = core_of[es] // 2
        gkey = ch * NB + bb_of[ed]
        o = np.argsort(gkey, kind="stable")
        gkey = gkey[o]
        wrow = (row_of[es] - ch * CHROWS)[o]
        pd = p_of[ed][o]
        cnt = np.bincount(gkey, minlength=NCH * NB)
        maxcnt = max(maxcnt, int(cnt.max()))
        edata.append((gkey, wrow, pd, cnt))
    T = max(512, ((maxcnt + 127) // 128) * 128)
    NSLOT = NCH * NB * T

    gidx = np.zeros((NC, 128, NSLOT // 16), np.int16)
    dstl = np.empty((NC, 128, NSLOT // 128), np.float32)
    for c in range(NC):
        gkey, wrow, pd, cnt = edata[c]
        cum = np.zeros(NCH * NB + 1, np.int64)
        cum[1:] = np.cumsum(cnt)
        slot = gkey * T + (np.arange(len(gkey)) - cum[gkey])
        gi = np.zeros(NSLOT, np.int16)
        gi[slot] = wrow.astype(np.int16)
        gidx[c] = np.tile(gi.reshape(-1, 16).T, (8, 1))
        dl = np.full(NSLOT, -1.0, np.float32)
        dl[slot] = pd.astype(np.float32)
        dstl[c] = np.ascontiguousarray(dl.reshape(-1, 128).T)
    return dis, bb_of, p_of, T, gidx, dstl


def _build_program(T):
    import os
    from concourse import bacc, bass, mybir
    import concourse.tile as tile

    no_gather = os.environ.get("KERNEL_NO_GATHER", "") == "1"
    no_coll = os.environ.get("KERNEL_NO_COLL", "") == "1"
    no_agg = os.environ.get("KERNEL_NO_AGG", "") == "1"
    no_trans = os.environ.get("KERNEL_NO_TRANS", "") == "1"
    nch_lim = int(os.environ.get("KERNEL_CHUNKS", str(NCH)))

    f32 = mybir.dt.float32
    i16 = mybir.dt.int16
    AF = mybir.ActivationFunctionType
    ALU = mybir.AluOpType
    NSLOT = NCH * NB * T
    M = T // 128
    GSUB = 1024  # dma_gather device limit: num_idxs <= 1024 per instruction

    nc = bacc.Bacc(None, target_bir_lowering=False)
    xT_h = nc.declare_dram_parameter("xT", [D, NPAD], f32, False)
    disc_h = nc.declare_dram_parameter("disc", [128, NB], f32, False)
    batc_h = nc.declare_dram_parameter("batc", [128, NB], f32, False)
    gidx_h = nc.declare_dram_parameter("gidx", [128, NSLOT // 16], i16, False)
    dstl_h = nc.declare_dram_parameter("dstl", [128, NSLOT // 128], f32, False)
    w_h = [nc.declare_dram_parameter(f"w{i}", [D, D], f32, False) for i in range(3)]
    b_h = [nc.declare_dram_parameter(f"b{i}", [128, D], f32, False) for i in range(3)]
    iota_h = nc.declare_dram_parameter("iota", [128, 128], f32, False)
    ident_h = nc.declare_dram_parameter("ident", [128, 128], f32, False)
    gid_h = nc.declare_dram_parameter("gid", [128, NG], f32, False)
    pooled_h = nc.declare_dram_parameter("pooled", [NG, D], f32, True)

    g_local = [
        nc.dram_tensor(f"g_local{L}", [128, NB * 64], f32, kind="Internal")
        for L in range(3)
    ]
    g_full = [
        nc.dram_tensor(
            f"g_full{L}", [NC * NPAD, 64], f32, kind="Internal", addr_space="Shared"
        )
        for L in range(3)
    ]
    if no_coll:
        tok_l = nc.dram_tensor("tok_l", [1, 64], f32, kind="Internal")
        tok_f = nc.dram_tensor(
            "tok_f", [8, 64], f32, kind="Internal", addr_space="Shared"
        )

    with tile.TileContext(nc) as tc:
        with tc.tile_pool(name="sb", bufs=1) as sb, tc.tile_pool(
            name="pp", bufs=1, space="PSUM"
        ) as pp:
            hT = sb.tile([D, NPAD], f32)
            nc.sync.dma_start(out=hT[:], in_=xT_h[:])
            dis_sb = sb.tile([128, NB], f32)
            nc.sync.dma_start(out=dis_sb[:], in_=disc_h[:])
            bat_sb = sb.tile([128, NB], f32)
            nc.sync.dma_start(out=bat_sb[:], in_=batc_h[:])
            gidx_sb = sb.tile([128, NSLOT // 16], i16)
            nc.sync.dma_start(out=gidx_sb[:], in_=gidx_h[:])
            dstl_sb = sb.tile([128, NSLOT // 128], f32)
            nc.sync.dma_start(out=dstl_sb[:], in_=dstl_h[:])
            w_sb, b_sb = [], []
            for i in range(3):
                wt = sb.tile([D, D], f32, name=f"w_sb{i}")
                nc.sync.dma_start(out=wt[:], in_=w_h[i][:])
                w_sb.append(wt)
                bt = sb.tile([128, D], f32, name=f"b_sb{i}")
                nc.sync.dma_start(out=bt[:], in_=b_h[i][:])
                b_sb.append(bt)
            iota_sb = sb.tile([128, 128], f32)
            nc.sync.dma_start(out=iota_sb[:], in_=iota_h[:])
            ident_sb = sb.tile([128, 128], f32)
            nc.sync.dma_start(out=ident_sb[:], in_=ident_h[:])
            gid_sb = sb.tile([128, NG], f32)
            nc.sync.dma_start(out=gid_sb[:], in_=gid_h[:])

            G_sb = sb.tile([128, NB * 64], f32)
            A_sb = sb.tile([128, NB * 64], f32)
            sub_cnts = sorted({min(GSUB, T - k * GSUB) for k in range((T + GSUB - 1) // GSUB)})
            sub_regs = {cnt: nc.gpsimd.to_reg(cnt) for cnt in sub_cnts}
            if no_coll:
                tok_sb = sb.tile([1, 64], f32)
                nc.vector.memset(tok_sb[:], 1.0)
                nc.sync.dma_start(out=tok_l[:], in_=tok_sb[:])
                nc.gpsimd.collective_compute(
                    "AllGather",
                    ALU.bypass,
                    replica_groups=RG,
                    ins=[tok_l[:]],
                    outs=[tok_f[:]],
                )

            for L in range(3):
                for bb in range(NB):
                    gps = pp.tile([128, D], f32, bufs=2)
                    nc.tensor.matmul(
                        out=gps[:],
                        lhsT=hT[:, bb * 128 : (bb + 1) * 128],
                        rhs=w_sb[L][:],
                        start=True,
                        stop=True,
                    )
                    nc.vector.tensor_tensor(
                        out=G_sb[:, bb * 64 : (bb + 1) * 64],
                        in0=gps[:],
                        in1=dis_sb[:, bb : bb + 1].to_broadcast([128, 64]),
                        op=ALU.mult,
                    )
                nc.sync.dma_start(out=g_local[L][:], in_=G_sb[:])
                if no_coll:
                    nc.sync.dma_start(
                        out=g_full[L][:128, :64], in_=g_local[L][:, :64]
                    )
                else:
                    nc.gpsimd.collective_compute(
                        "AllGather",
                        ALU.bypass,
                        replica_groups=RG,
                        ins=[g_local[L][:]],
                        outs=[g_full[L][:]],
                    )
                if no_agg:
                    nc.vector.memset(A_sb[:], 0.0)
                for ch in range(nch_lim) if not no_agg else []:
                    win = g_full[L][ch * CHROWS : (ch + 1) * CHROWS, :]
                    for bb in range(NB):
                        msgs = sb.tile([128, T // 128, 64], f32, bufs=3)
                        s0 = ch * NB * T + bb * T
                        if no_gather:
                            nc.vector.memset(msgs[:], 0.0)
                        else:
                            for k in range(0, T, GSUB):
                                cnt = min(GSUB, T - k)
                                nc.gpsimd.dma_gather(
                                    out_ap=msgs[
                                        :, k // 128 : (k + cnt) // 128, :
                                    ],
                                    in_ap=win,
                                    idxs_ap=gidx_sb[
                                        :, (s0 + k) // 16 : (s0 + k + cnt) // 16
                                    ],
                                    num_idxs=cnt,
                                    num_idxs_reg=sub_regs[cnt],
                                    elem_size=64,
                                )
                        aps = pp.tile([128, D], f32, bufs=3)
                        for m in range(M):
                            sel = sb.tile([128, 128], f32, bufs=4)
                            dcol = s0 // 128 + m
                            nc.vector.tensor_tensor(
                                out=sel[:],
                                in0=dstl_sb[:, dcol : dcol + 1].to_broadcast(
                                    [128, 128]
                                ),
                                in1=iota_sb[:],
                                op=ALU.is_equal,
                            )
                            nc.tensor.matmul(
                                out=aps[:],
                                lhsT=sel[:],
                                rhs=msgs[:, m, :],
                                start=(m == 0),
                                stop=(m == M - 1),
                            )
                        bsl = slice(bb * 64, (bb + 1) * 64)
                        if ch == 0:
                            nc.scalar.activation(
                                out=A_sb[:, bsl], in_=aps[:], func=AF.Copy
                            )
                        else:
                            nc.vector.tensor_add(
                                out=A_sb[:, bsl], in0=A_sb[:, bsl], in1=aps[:]
                            )
                for bb in range(NB):
                    bsl = slice(bb * 64, (bb + 1) * 64)
                    nc.scalar.activation(
                        out=A_sb[:, bsl],
                        in_=A_sb[:, bsl],
                        func=AF.Copy,
                        scale=dis_sb[:, bb : bb + 1],
                    )
                    nc.vector.tensor_add(
                        out=G_sb[:, bsl], in0=A_sb[:, bsl], in1=b_sb[L][:]
                    )
                    nc.scalar.activation(
                        out=G_sb[:, bsl], in_=G_sb[:, bsl], func=AF.Relu
                    )
                if L < 2:
                    for bb in range(NB):
                        if no_trans:
                            nc.scalar.activation(
                                out=hT[:64, bb * 128 : bb * 128 + 64],
                                in_=G_sb[:64, bb * 64 : (bb + 1) * 64],
                                func=AF.Copy,
                            )
                            continue
                        tps = pp.tile([D, 128], f32, bufs=2)
                        nc.tensor.transpose(
                            out=tps[:],
                            in_=G_sb[:, bb * 64 : (bb + 1) * 64],
                            identity=ident_sb[:],
                        )
                        nc.scalar.activation(
                            out=hT[:, bb * 128 : (bb + 1) * 128],
                            in_=tps[:],
                            func=AF.Copy,
                        )
                else:
                    pps = pp.tile([NG, D], f32)
                    for bb in range(NB):
                        oh = sb.tile([128, NG], f32, bufs=2)
                        nc.vector.tensor_tensor(
                            out=oh[:],
                            in0=bat_sb[:, bb : bb + 1].to_broadcast([128, NG]),
                            in1=gid_sb[:],
                            op=ALU.is_equal,
                        )
                        nc.tensor.matmul(
                            out=pps[:],
                            lhsT=oh[:],
                            rhs=G_sb[:, bb * 64 : (bb + 1) * 64],
                            start=(bb == 0),
                            stop=(bb == NB - 1),
                        )
                    pool_sb = sb.tile([NG, D], f32)
                    nc.scalar.activation(out=pool_sb[:], in_=pps[:], func=AF.Copy)
                    nc.sync.dma_start(out=pooled_h[:], in_=pool_sb[:])
    if not nc.is_finalized():
        nc.finalize()
    return nc


LAST_RESULTS = None


def kernel(**inputs):
    from concourse.bass_utils import run_bass_kernel_spmd

    x = np.asarray(inputs["x"], np.float32)
    edge_index = np.asarray(inputs["edge_index"])
    batch = np.asarray(inputs["batch"])
    W = [np.asarray(inputs[k], np.float32) for k in ("W1", "W2", "W3")]
    b = [np.asarray(inputs[k], np.float32) for k in ("b1", "b2", "b3")]
    lin_w = np.asarray(inputs["lin_w"], np.float32)
    lin_b = np.asarray(inputs["lin_b"], np.float32)

    dis, bb_of, p_of, T, gidx, dstl = _preprocess(edge_index)

    xT = np.zeros((NC, D, NPAD), np.float32)
    disc = np.zeros((NC, 128, NB), np.float32)
    batc = np.full((NC, 128, NB), -1.0, np.float32)
    for c in range(NC):
        nodes = np.arange(c * NPC, (c + 1) * NPC)
        col = bb_of[nodes] * 128 + p_of[nodes]
        xT[c][:, col] = x[nodes].T
        disc[c][p_of[nodes], bb_of[nodes]] = dis[nodes]
        batc[c][p_of[nodes], bb_of[nodes]] = batch[nodes].astype(np.float32)

    iota = np.ascontiguousarray(np.tile(np.arange(128, dtype=np.float32), (128, 1)))
    ident = np.eye(128, dtype=np.float32)
    gid = np.ascontiguousarray(np.tile(np.arange(NG, dtype=np.float32), (128, 1)))
    b_repl = [np.ascontiguousarray(np.tile(bi.reshape(1, D), (128, 1))) for bi in b]

    nc = _build_program(T)
    in_maps = []
    for c in range(NC):
        in_maps.append(
            {
                "xT": np.ascontiguousarray(xT[c]),
                "disc": np.ascontiguousarray(disc[c]),
                "batc": np.ascontiguousarray(batc[c]),
                "gidx": np.ascontiguousarray(gidx[c]),
                "dstl": np.ascontiguousarray(dstl[c]),
                "w0": W[0],
                "w1": W[1],
                "w2": W[2],
                "b0": b_repl[0],
                "b1": b_repl[1],
                "b2": b_repl[2],
                "iota": iota,
                "ident": ident,
                "gid": gid,
            }
        )
    import os

    trace = os.environ.get("KERNEL_TRACE", "") == "1"
    res = run_bass_kernel_spmd(nc, in_maps, list(range(NC)), trace=trace)
    global LAST_RESULTS
    LAST_RESULTS = res
    pooled = np.zeros((NG, D), np.float64)
    for r in res.results:
        pooled += r["pooled"].astype(np.float64)
    out = pooled.astype(np.float32) @ lin_w + lin_b
    return out.astype(np.float32)



# revision 2
# speedup vs baseline: 1.0187x; 1.0187x over previous
import numpy as np

N = 100000
D = 64
NG = 64
NC = 8
NPC = N // NC          # 12500 real nodes per core
NB = 98                # blocks of 128 dst nodes per core
NPAD = NB * 128        # 12544 padded nodes per core
NCH = 4                # src chunks (2 cores each)
CHROWS = 2 * NPAD      # 25088 table rows per chunk (< int16 max)
GSUB = 1024            # max idxs per dma_gather instruction
RG = [[0, 1, 2, 3, 4, 5, 6, 7]]


def _preprocess(edge_index):
    # real edges only; self-loops handled on-device (A += G)
    src = edge_index[0].astype(np.int64)
    dst = edge_index[1].astype(np.int64)
    deg = np.bincount(dst, minlength=N) + 1  # +1 for self loop
    dis = (1.0 / np.sqrt(deg.astype(np.float64))).astype(np.float32)

    core_of = np.arange(N) // NPC
    bb_of = np.empty(N, np.int64)
    p_of = np.empty(N, np.int64)
    for c in range(NC):
        nodes = np.arange(c * NPC, (c + 1) * NPC)
        order = np.argsort(-deg[nodes], kind="stable")
        r = np.empty(NPC, np.int64)
        r[order] = np.arange(NPC)
        bb_of[nodes] = r % NB
        p_of[nodes] = r // NB
    row_of = core_of * NPAD + p_of * NB + bb_of

    NGRP = NCH * NB
    # per-core group data
    edata = []
    cnts = np.zeros((NC, NGRP), np.int64)
    dst_core = core_of[dst]
    for c in range(NC):
        m = dst_core == c
        es, ed = src[m], dst[m]
        ch = core_of[es] // 2
        gkey = ch * NB + bb_of[ed]
        wrow = row_of[es] - ch * CHROWS
        o = np.lexsort((wrow, gkey))  # group-major, src-row ascending inside
        gkey, wrow, pd = gkey[o], wrow[o], p_of[ed][o]
        cnts[c] = np.bincount(gkey, minlength=NGRP)
        edata.append((gkey, wrow, pd))

    # shared static group sizes: max over cores, rounded to 16
    gsz = ((cnts.max(axis=0) + 15) // 16) * 16

    # chunk-local slot streams: groups back-to-back (16-aligned), windows of 1024
    grp_off = np.zeros(NGRP, np.int64)     # slot offset within chunk stream
    ch_len = np.zeros(NCH, np.int64)
    for ch in range(NCH):
        off = 0
        for bb in range(NB):
            g = ch * NB + bb
            grp_off[g] = off
            off += gsz[g]
        ch_len[ch] = off
    # pad each chunk stream to multiple of 128 (for gather out tiles)
    ch_len128 = ((ch_len + 127) // 128) * 128
    ch_base = np.zeros(NCH + 1, np.int64)
    ch_base[1:] = np.cumsum(ch_len128)
    NSLOT = int(ch_base[-1])

    # label columns: per chunk column, 1 or 2 (group, labelcol) entries
    # build per-group list of (col_in_chunk, labelcol_index)
    n_ch_cols = (ch_len128 // 128).astype(np.int64)
    grp_cols = []   # per group: list of (tile_col_global, labelcol)
    lc_count = 0
    for ch in range(NCH):
        for bb in range(NB):
            g = ch * NB + bb
            s0, s1 = grp_off[g], grp_off[g] + gsz[g]
            c0, c1 = s0 // 128, (s1 - 1) // 128
            cols = []
            for cc in range(c0, c1 + 1):
                cols.append((int(ch_base[ch]) // 128 + cc, lc_count))
                lc_count += 1
            grp_cols.append(cols)
    NLC = lc_count

    # build per-core gidx (int16 idx stream) and dstl (label columns)
    gidx = np.zeros((NC, 128, NSLOT // 16), np.int16)
    dstl = np.empty((NC, 128, NLC), np.float32)
    for c in range(NC):
        gkey, wrow, pd = edata[c]
        gi = np.zeros(NSLOT, np.int16)
        lab = np.full((NSLOT,), -1.0, np.float32)   # slot label (p_of), -1 pad
        gstart = np.zeros(NGRP + 1, np.int64)
        gstart[1:] = np.cumsum(cnts[c])
        for ch in range(NCH):
            for bb in range(NB):
                g = ch * NB + bb
                a, b = gstart[g], gstart[g + 1]
                base = int(ch_base[ch] + grp_off[g])
                n = b - a
                gi[base : base + n] = wrow[a:b].astype(np.int16)
                lab[base : base + n] = pd[a:b].astype(np.float32)
                # dummy pads: valid idx 0, label stays -1
        gidx[c] = np.tile(gi.reshape(-1, 16).T, (8, 1))
        # label columns: for each group column, mask to that group's slot range
        dl = np.empty((128, NLC), np.float32)
        for g in range(NGRP):
            ch = g // NB
            s0 = int(ch_base[ch] + grp_off[g])
            s1 = s0 + int(gsz[g])
            for (tcol, lc) in grp_cols[g]:
                colbase = tcol * 128
                col = np.full(128, -1.0, np.float32)
                lo = max(s0, colbase)
                hi = min(s1, colbase + 128)
                col[lo - colbase : hi - colbase] = lab[lo:hi]
                dl[:, lc] = col
        dstl[c] = dl
    return dis, bb_of, p_of, gsz, grp_cols, ch_base, n_ch_cols, gidx, dstl


def _build_program(gsz, grp_cols, ch_base, n_ch_cols, NSLOT, NLC):
    from concourse import bacc, mybir

    import concourse.tile as tile

    f32 = mybir.dt.float32
    i16 = mybir.dt.int16
    AF = mybir.ActivationFunctionType
    ALU = mybir.AluOpType

    nc = bacc.Bacc(None, target_bir_lowering=False)
    xT_h = nc.declare_dram_parameter("xT", [D, NPAD], f32, False)
    disc_h = nc.declare_dram_parameter("disc", [128, NB], f32, False)
    batc_h = nc.declare_dram_parameter("batc", [128, NB], f32, False)
    gidx_h = nc.declare_dram_parameter("gidx", [128, NSLOT // 16], i16, False)
    dstl_h = nc.declare_dram_parameter("dstl", [128, NLC], f32, False)
    w_h = [nc.declare_dram_parameter(f"w{i}", [D, D], f32, False) for i in range(3)]
    b_h = [nc.declare_dram_parameter(f"b{i}", [128, D], f32, False) for i in range(3)]
    iota_h = nc.declare_dram_parameter("iota", [128, 128], f32, False)
    ident_h = nc.declare_dram_parameter("ident", [128, 128], f32, False)
    gid_h = nc.declare_dram_parameter("gid", [128, NG], f32, False)
    pooled_h = nc.declare_dram_parameter("pooled", [NG, D], f32, True)

    g_local = [
        nc.dram_tensor(f"g_local{L}", [128, NB * 64], f32, kind="Internal")
        for L in range(3)
    ]
    g_full = [
        nc.dram_tensor(
            f"g_full{L}", [NC * NPAD, 64], f32, kind="Internal", addr_space="Shared"
        )
        for L in range(3)
    ]

    NGRP = NCH * NB

    with tile.TileContext(nc) as tc:
        with tc.tile_pool(name="sb", bufs=1) as sb, tc.tile_pool(
            name="pp", bufs=1, space="PSUM"
        ) as pp:
            hT = sb.tile([D, NPAD], f32)
            nc.sync.dma_start(out=hT[:], in_=xT_h[:])
            dis_sb = sb.tile([128, NB], f32)
            nc.sync.dma_start(out=dis_sb[:], in_=disc_h[:])
            bat_sb = sb.tile([128, NB], f32)
            nc.sync.dma_start(out=bat_sb[:], in_=batc_h[:])
            gidx_sb = sb.tile([128, NSLOT // 16], i16)
            nc.sync.dma_start(out=gidx_sb[:], in_=gidx_h[:])
            dstl_sb = sb.tile([128, NLC], f32)
            nc.sync.dma_start(out=dstl_sb[:], in_=dstl_h[:])
            w_sb, b_sb = [], []
            for i in range(3):
                wt = sb.tile([D, D], f32, name=f"w_sb{i}")
                nc.sync.dma_start(out=wt[:], in_=w_h[i][:])
                w_sb.append(wt)
                bt = sb.tile([128, D], f32, name=f"b_sb{i}")
                nc.sync.dma_start(out=bt[:], in_=b_h[i][:])
                b_sb.append(bt)
            iota_sb = sb.tile([128, 128], f32)
            nc.sync.dma_start(out=iota_sb[:], in_=iota_h[:])
            ident_sb = sb.tile([128, 128], f32)
            nc.sync.dma_start(out=ident_sb[:], in_=ident_h[:])
            gid_sb = sb.tile([128, NG], f32)
            nc.sync.dma_start(out=gid_sb[:], in_=gid_h[:])

            G_sb = sb.tile([128, NB * 64], f32)
            A_sb = sb.tile([128, NB * 64], f32)
            reg1024 = nc.gpsimd.to_reg(GSUB)
            tail_regs = {}

            for L in range(3):
                # ---- transform: G = (h @ W) * dis ----
                for bb in range(NB):
                    gps = pp.tile([128, D], f32, bufs=2)
                    nc.tensor.matmul(
                        out=gps[:],
                        lhsT=hT[:, bb * 128 : (bb + 1) * 128],
                        rhs=w_sb[L][:],
                        start=True,
                        stop=True,
                    )
                    nc.vector.tensor_tensor(
                        out=G_sb[:, bb * 64 : (bb + 1) * 64],
                        in0=gps[:],
                        in1=dis_sb[:, bb : bb + 1].to_broadcast([128, 64]),
                        op=ALU.mult,
                    )
                nc.sync.dma_start(out=g_local[L][:], in_=G_sb[:])
                nc.gpsimd.collective_compute(
                    "AllGather",
                    ALU.bypass,
                    replica_groups=RG,
                    ins=[g_local[L][:]],
                    outs=[g_full[L][:]],
                )

                # ---- gathers: per chunk, 1024-idx windows ----
                # msgs tiles: one per window, bufs=6
                msgs_of_col = {}  # global tile col -> (tile, sub)
                for ch in range(NCH):
                    win = g_full[L][ch * CHROWS : (ch + 1) * CHROWS, :]
                    nslots_ch = int(n_ch_cols[ch]) * 128
                    base = int(ch_base[ch])
                    for k in range(0, nslots_ch, GSUB):
                        cnt = min(GSUB, nslots_ch - k)
                        if cnt not in tail_regs:
                            tail_regs[cnt] = (
                                reg1024 if cnt == GSUB else nc.gpsimd.to_reg(cnt)
                            )
                        mt = sb.tile([128, GSUB // 128, 64], f32, name="msgs", bufs=6)
                        nc.gpsimd.dma_gather(
                            out_ap=mt[:, : cnt // 128, :],
                            in_ap=win,
                            idxs_ap=gidx_sb[
                                :, (base + k) // 16 : (base + k + cnt) // 16
                            ],
                            num_idxs=cnt,
                            num_idxs_reg=tail_regs[cnt],
                            elem_size=64,
                        )
                        for j in range(cnt // 128):
                            msgs_of_col[(base + k) // 128 + j] = (mt, j)

                # ---- scatter: per group, masked-sel matmuls ----
                for g in range(NGRP):
                    ch, bb = g // NB, g % NB
                    cols = grp_cols[g]
                    ncols = len(cols)
                    sel = sb.tile([128, 8 * 128], f32, name="sel", bufs=4)
                    for ci, (tcol, lc) in enumerate(cols):
                        nc.vector.tensor_tensor(
                            out=sel[:, ci * 128 : (ci + 1) * 128],
                            in0=dstl_sb[:, lc : lc + 1].to_broadcast([128, 128]),
                            in1=iota_sb[:],
                            op=ALU.is_equal,
                        )
                    aps = pp.tile([128, D], f32, bufs=3)
                    for ci, (tcol, lc) in enumerate(cols):
                        mt, j = msgs_of_col[tcol]
                        nc.tensor.matmul(
                            out=aps[:],
                            lhsT=sel[:, ci * 128 : (ci + 1) * 128],
                            rhs=mt[:, j, :],
                            start=(ci == 0),
                            stop=(ci == ncols - 1),
                        )
                    bsl = slice(bb * 64, (bb + 1) * 64)
                    if ch == 0:
                        nc.scalar.activation(
                            out=A_sb[:, bsl], in_=aps[:], func=AF.Copy
                        )
                    else:
                        nc.vector.tensor_add(
                            out=A_sb[:, bsl], in0=A_sb[:, bsl], in1=aps[:]
                        )

                # ---- self-loops + scale + bias + relu ----
                nc.vector.tensor_add(out=A_sb[:], in0=A_sb[:], in1=G_sb[:])
                for bb in range(NB):
                    bsl = slice(bb * 64, (bb + 1) * 64)
                    nc.scalar.activation(
                        out=A_sb[:, bsl],
                        in_=A_sb[:, bsl],
                        func=AF.Copy,
                        scale=dis_sb[:, bb : bb + 1],
                    )
                    nc.vector.tensor_add(
                        out=G_sb[:, bsl], in0=A_sb[:, bsl], in1=b_sb[L][:]
                    )
                    nc.scalar.activation(
                        out=G_sb[:, bsl], in_=G_sb[:, bsl], func=AF.Relu
                    )
                if L < 2:
                    for bb in range(NB):
                        tps = pp.tile([D, 128], f32, bufs=2)
                        nc.tensor.transpose(
                            out=tps[:],
                            in_=G_sb[:, bb * 64 : (bb + 1) * 64],
                            identity=ident_sb[:],
                        )
                        nc.scalar.activation(
                            out=hT[:, bb * 128 : (bb + 1) * 128],
                            in_=tps[:],
                            func=AF.Copy,
                        )
                else:
                    pps = pp.tile([NG, D], f32)
                    for bb in range(NB):
                        oh = sb.tile([128, NG], f32, bufs=2)
                        nc.vector.tensor_tensor(
                            out=oh[:],
                            in0=bat_sb[:, bb : bb + 1].to_broadcast([128, NG]),
                            in1=gid_sb[:],
                            op=ALU.is_equal,
                        )
                        nc.tensor.matmul(
                            out=pps[:],
                            lhsT=oh[:],
                            rhs=G_sb[:, bb * 64 : (bb + 1) * 64],
                            start=(bb == 0),
                            stop=(bb == NB - 1),
                        )
                    pool_sb = sb.tile([NG, D], f32)
                    nc.scalar.activation(out=pool_sb[:], in_=pps[:], func=AF.Copy)
                    nc.sync.dma_start(out=pooled_h[:], in_=pool_sb[:])
    if not nc.is_finalized():
        nc.finalize()
    return nc


LAST_RESULTS = None


def kernel(**inputs):
    from concourse.bass_utils import run_bass_kernel_spmd

    x = np.asarray(inputs["x"], np.float32)
    edge_index = np.asarray(inputs["edge_index"])
    batch = np.asarray(inputs["batch"])
    W = [np.asarray(inputs[k], np.float32) for k in ("W1", "W2", "W3")]
    b = [np.asarray(inputs[k], np.float32) for k in ("b1", "b2", "b3")]
    lin_w = np.asarray(inputs["lin_w"], np.float32)
    lin_b = np.asarray(inputs["lin_b"], np.float32)

    dis, bb_of, p_of, gsz, grp_cols, ch_base, n_ch_cols, gidx, dstl = _preprocess(
        edge_index
    )
    NSLOT = int(ch_base[-1])
    NLC = dstl.shape[2]

    xT = np.zeros((NC, D, NPAD), np.float32)
    disc = np.zeros((NC, 128, NB), np.float32)
    batc = np.full((NC, 128, NB), -1.0, np.float32)
    for c in range(NC):
        nodes = np.arange(c * NPC, (c + 1) * NPC)
        col = bb_of[nodes] * 128 + p_of[nodes]
        xT[c][:, col] = x[nodes].T
        disc[c][p_of[nodes], bb_of[nodes]] = dis[nodes]
        batc[c][p_of[nodes], bb_of[nodes]] = batch[nodes].astype(np.float32)

    iota = np.ascontiguousarray(np.tile(np.arange(128, dtype=np.float32), (128, 1)))
    ident = np.eye(128, dtype=np.float32)
    gid = np.ascontiguousarray(np.tile(np.arange(NG, dtype=np.float32), (128, 1)))
    b_repl = [np.ascontiguousarray(np.tile(bi.reshape(1, D), (128, 1))) for bi in b]

    nc = _build_program(gsz, grp_cols, ch_base, n_ch_cols, NSLOT, NLC)
    in_maps = []
    for c in range(NC):
        in_maps.append(
            {
                "xT": np.ascontiguousarray(xT[c]),
                "disc": np.ascontiguousarray(disc[c]),
                "batc": np.ascontiguousarray(batc[c]),
                "gidx": np.ascontiguousarray(gidx[c]),
                "dstl": np.ascontiguousarray(dstl[c]),
                "w0": W[0],
                "w1": W[1],
                "w2": W[2],
                "b0": b_repl[0],
                "b1": b_repl[1],
                "b2": b_repl[2],
                "iota": iota,
                "ident": ident,
                "gid": gid,
            }
        )
    import os

    trace = os.environ.get("KERNEL_TRACE", "") == "1"
    res = run_bass_kernel_spmd(nc, in_maps, list(range(NC)), trace=trace)
    global LAST_RESULTS
    LAST_RESULTS = res
    pooled = np.zeros((NG, D), np.float64)
    for r in res.results:
        pooled += r["pooled"].astype(np.float64)
    out = pooled.astype(np.float32) @ lin_w + lin_b
    return out.astype(np.float32)


# revision 3
# speedup vs baseline: 1.0195x; 1.0008x over previous
import numpy as np

N = 100000
D = 64
NG = 64
NC = 8
NPC = N // NC          # 12500 real nodes per core
NB = 98                # blocks of 128 dst nodes per core
NBA = 49               # blocks in each table half
NPAD = NB * 128        # 12544 padded nodes per core
HROWS = NBA * 128      # 6272 rows per core per half-table
NCH = 4                # src chunks (2 cores each)
WROWS = 2 * HROWS      # 12544 rows per (chunk, half) gather window
GSUB = 1024            # max idxs per dma_gather instruction
RG = [[0, 1, 2, 3, 4, 5, 6, 7]]
NST = NCH * 2          # gather streams: (chunk, src-half)
NGRP = NST * NB


def _preprocess(edge_index):
    src = edge_index[0].astype(np.int64)
    dst = edge_index[1].astype(np.int64)
    deg = np.bincount(dst, minlength=N) + 1  # +1 self loop
    dis = (1.0 / np.sqrt(deg.astype(np.float64))).astype(np.float32)

    core_of = np.arange(N) // NPC
    bb_of = np.empty(N, np.int64)
    p_of = np.empty(N, np.int64)
    for c in range(NC):
        nodes = np.arange(c * NPC, (c + 1) * NPC)
        order = np.argsort(-deg[nodes], kind="stable")
        r = np.empty(NPC, np.int64)
        r[order] = np.arange(NPC)
        bb_of[nodes] = r % NB
        p_of[nodes] = r // NB

    # half-table row: half = bb>=NBA; row(core,half) = (core%2)*HROWS + p*NBA + bb_h
    half_of = (bb_of >= NBA).astype(np.int64)
    bb_h = bb_of - half_of * NBA
    wrow_of = (core_of % 2) * HROWS + p_of * NBA + bb_h

    edata = []
    cnts = np.zeros((NC, NGRP), np.int64)
    dst_core = core_of[dst]
    for c in range(NC):
        m = dst_core == c
        es, ed = src[m], dst[m]
        st = (core_of[es] // 2) * 2 + half_of[es]     # stream = (chunk, half)
        gkey = st * NB + bb_of[ed]
        wrow = wrow_of[es]
        o = np.lexsort((wrow, gkey))
        gkey, wrow, pd = gkey[o], wrow[o], p_of[ed][o]
        cnts[c] = np.bincount(gkey, minlength=NGRP)
        edata.append((gkey, wrow, pd))

    gsz = ((cnts.max(axis=0) + 15) // 16) * 16

    grp_off = np.zeros(NGRP, np.int64)
    st_len = np.zeros(NST, np.int64)
    for s in range(NST):
        off = 0
        for bb in range(NB):
            g = s * NB + bb
            grp_off[g] = off
            off += gsz[g]
        st_len[s] = off
    st_len128 = ((st_len + 127) // 128) * 128
    st_base = np.zeros(NST + 1, np.int64)
    st_base[1:] = np.cumsum(st_len128)
    NSLOT = int(st_base[-1])
    n_st_cols = (st_len128 // 128).astype(np.int64)

    grp_cols = []
    lc_count = 0
    for s in range(NST):
        for bb in range(NB):
            g = s * NB + bb
            s0, s1 = grp_off[g], grp_off[g] + gsz[g]
            c0, c1 = s0 // 128, (s1 - 1) // 128
            cols = []
            for cc in range(c0, c1 + 1):
                cols.append((int(st_base[s]) // 128 + cc, lc_count))
                lc_count += 1
            grp_cols.append(cols)
    NLC = lc_count

    gidx = np.zeros((NC, 128, NSLOT // 16), np.int16)
    dstl = np.empty((NC, 128, NLC), np.float32)
    for c in range(NC):
        gkey, wrow, pd = edata[c]
        gi = np.zeros(NSLOT, np.int16)
        lab = np.full((NSLOT,), -1.0, np.float32)
        gstart = np.zeros(NGRP + 1, np.int64)
        gstart[1:] = np.cumsum(cnts[c])
        for g in range(NGRP):
            a, b = gstart[g], gstart[g + 1]
            base = int(st_base[g // NB] + grp_off[g])
            n = b - a
            gi[base : base + n] = wrow[a:b].astype(np.int16)
            lab[base : base + n] = pd[a:b].astype(np.float32)
        gidx[c] = np.tile(gi.reshape(-1, 16).T, (8, 1))
        dl = np.empty((128, NLC), np.float32)
        for g in range(NGRP):
            s0 = int(st_base[g // NB] + grp_off[g])
            s1 = s0 + int(gsz[g])
            for (tcol, lc) in grp_cols[g]:
                colbase = tcol * 128
                col = np.full(128, -1.0, np.float32)
                lo, hi = max(s0, colbase), min(s1, colbase + 128)
                col[lo - colbase : hi - colbase] = lab[lo:hi]
                dl[:, lc] = col
        dstl[c] = dl
    return dis, bb_of, p_of, grp_cols, st_base, n_st_cols, gidx, dstl


def _build_program(grp_cols, st_base, n_st_cols, NSLOT, NLC):
    from concourse import bacc, mybir

    import concourse.tile as tile

    f32 = mybir.dt.float32
    i16 = mybir.dt.int16
    AF = mybir.ActivationFunctionType
    ALU = mybir.AluOpType

    nc = bacc.Bacc(None, target_bir_lowering=False)
    xT_h = nc.declare_dram_parameter("xT", [D, NPAD], f32, False)
    disc_h = nc.declare_dram_parameter("disc", [128, NB], f32, False)
    batc_h = nc.declare_dram_parameter("batc", [128, NB], f32, False)
    gidx_h = nc.declare_dram_parameter("gidx", [128, NSLOT // 16], i16, False)
    dstl_h = nc.declare_dram_parameter("dstl", [128, NLC], f32, False)
    w_h = [nc.declare_dram_parameter(f"w{i}", [D, D], f32, False) for i in range(3)]
    br_h = [
        nc.declare_dram_parameter(f"br{i}", [128, NB * 64], f32, False)
        for i in range(3)
    ]
    iota_h = nc.declare_dram_parameter("iota", [128, 128], f32, False)
    ident_h = nc.declare_dram_parameter("ident", [128, 128], f32, False)
    gid_h = nc.declare_dram_parameter("gid", [128, NG], f32, False)
    pooled_h = nc.declare_dram_parameter("pooled", [NG, D], f32, True)

    g_loc = [
        [
            nc.dram_tensor(f"g_loc{L}_{h}", [128, NBA * 64], f32, kind="Internal")
            for h in range(2)
        ]
        for L in range(3)
    ]
    g_ful = [
        [
            nc.dram_tensor(
                f"g_ful{L}_{h}",
                [NC * HROWS, 64],
                f32,
                kind="Internal",
                addr_space="Shared",
            )
            for h in range(2)
        ]
        for L in range(3)
    ]

    from concourse import library_config

    with tile.TileContext(nc) as tc:
        with tc.tile_pool(name="sb", bufs=1) as sb, tc.tile_pool(
            name="pp", bufs=1, space="PSUM"
        ) as pp:
            nc.gpsimd.load_library(library_config.mlp)
            hT = sb.tile([D, NPAD], f32)
            nc.sync.dma_start(out=hT[:], in_=xT_h[:])
            dis_sb = sb.tile([128, NB], f32)
            nc.sync.dma_start(out=dis_sb[:], in_=disc_h[:])
            bat_sb = sb.tile([128, NB], f32)
            nc.sync.dma_start(out=bat_sb[:], in_=batc_h[:])
            gidx_sb = sb.tile([128, NSLOT // 16], i16)
            nc.sync.dma_start(out=gidx_sb[:], in_=gidx_h[:])
            dstl_sb = sb.tile([128, NLC], f32)
            nc.sync.dma_start(out=dstl_sb[:], in_=dstl_h[:])
            w_sb = []
            for i in range(3):
                wt = sb.tile([D, D], f32, name=f"w_sb{i}")
                nc.sync.dma_start(out=wt[:], in_=w_h[i][:])
                w_sb.append(wt)
            br_sb = sb.tile([128, NB * 64], f32)
            iota_sb = sb.tile([128, 128], f32)
            nc.sync.dma_start(out=iota_sb[:], in_=iota_h[:])
            ident_sb = sb.tile([128, 128], f32)
            nc.sync.dma_start(out=ident_sb[:], in_=ident_h[:])
            gid_sb = sb.tile([128, NG], f32)
            nc.sync.dma_start(out=gid_sb[:], in_=gid_h[:])

            G_sb = sb.tile([128, NB * 64], f32)
            A_sb = sb.tile([128, NB * 64], f32)
            reg1024 = nc.gpsimd.to_reg(GSUB)
            tail_regs = {GSUB: reg1024}
            HS = [slice(0, NBA * 64), slice(NBA * 64, NB * 64)]

            for L in range(3):
                # transform + per-half table publish
                for h in range(2):
                    for bb in range(h * NBA, (h + 1) * NBA):
                        gps = pp.tile([128, D], f32, bufs=2)
                        nc.tensor.matmul(
                            out=gps[:],
                            lhsT=hT[:, bb * 128 : (bb + 1) * 128],
                            rhs=w_sb[L][:],
                            start=True,
                            stop=True,
                        )
                        nc.vector.tensor_tensor(
                            out=G_sb[:, bb * 64 : (bb + 1) * 64],
                            in0=gps[:],
                            in1=dis_sb[:, bb : bb + 1].to_broadcast([128, 64]),
                            op=ALU.mult,
                        )
                    nc.sync.dma_start(out=g_loc[L][h][:], in_=G_sb[:, HS[h]])
                    nc.gpsimd.collective_compute(
                        "AllGather",
                        ALU.bypass,
                        replica_groups=RG,
                        ins=[g_loc[L][h][:]],
                        outs=[g_ful[L][h][:]],
                    )

                # gathers: per (chunk, half) stream, 1024-idx windows
                msgs_of_col = {}
                for s in range(NST):
                    ch, h = s // 2, s % 2
                    win = g_ful[L][h][ch * WROWS : (ch + 1) * WROWS, :]
                    nslots = int(n_st_cols[s]) * 128
                    base = int(st_base[s])
                    for k in range(0, nslots, GSUB):
                        cnt = min(GSUB, nslots - k)
                        if cnt not in tail_regs:
                            tail_regs[cnt] = nc.gpsimd.to_reg(cnt)
                        mt = sb.tile([128, GSUB // 128, 64], f32, name="msgs", bufs=8)
                        nc.gpsimd.dma_gather(
                            out_ap=mt[:, : cnt // 128, :],
                            in_ap=win,
                            idxs_ap=gidx_sb[
                                :, (base + k) // 16 : (base + k + cnt) // 16
                            ],
                            num_idxs=cnt,
                            num_idxs_reg=tail_regs[cnt],
                            elem_size=64,
                        )
                        for j in range(cnt // 128):
                            msgs_of_col[(base + k) // 128 + j] = (mt, j)

                # scatter
                for g in range(NGRP):
                    s, bb = g // NB, g % NB
                    cols = grp_cols[g]
                    ncols = len(cols)
                    sel = sb.tile([128, 8 * 128], f32, name="sel", bufs=6)
                    for ci, (tcol, lc) in enumerate(cols):
                        nc.vector.tensor_tensor(
                            out=sel[:, ci * 128 : (ci + 1) * 128],
                            in0=dstl_sb[:, lc : lc + 1].to_broadcast([128, 128]),
                            in1=iota_sb[:],
                            op=ALU.is_equal,
                        )
                    aps = pp.tile([128, D], f32, bufs=3)
                    for ci, (tcol, lc) in enumerate(cols):
                        mt, j = msgs_of_col[tcol]
                        nc.tensor.matmul(
                            out=aps[:],
                            lhsT=sel[:, ci * 128 : (ci + 1) * 128],
                            rhs=mt[:, j, :],
                            start=(ci == 0),
                            stop=(ci == ncols - 1),
                        )
                    bsl = slice(bb * 64, (bb + 1) * 64)
                    if s == 0:
                        nc.scalar.activation(
                            out=A_sb[:, bsl], in_=aps[:], func=AF.Copy
                        )
                    else:
                        nc.vector.tensor_add(
                            out=A_sb[:, bsl], in0=A_sb[:, bsl], in1=aps[:]
                        )

                # epilogue per dst half: self-loop, scale, bias, relu
                nc.sync.dma_start(out=br_sb[:], in_=br_h[L][:])
                for h in range(2):
                    hs = HS[h]
                    dsl = dis_sb[:, h * NBA : (h + 1) * NBA]
                    nc.vector.tensor_add(
                        out=A_sb[:, hs], in0=A_sb[:, hs], in1=G_sb[:, hs]
                    )
                    nc.vector.tensor_tensor(
                        out=A_sb[:, hs],
                        in0=A_sb[:, hs],
                        in1=dsl.to_broadcast([128, NBA, 64]),
                        op=ALU.mult,
                    )
                    nc.vector.tensor_add(
                        out=G_sb[:, hs], in0=A_sb[:, hs], in1=br_sb[:, hs]
                    )
                    nc.scalar.activation(
                        out=G_sb[:, hs], in_=G_sb[:, hs], func=AF.Relu
                    )
                    if L < 2:
                        for bb in range(h * NBA, (h + 1) * NBA):
                            tps = pp.tile([D, 128], f32, bufs=2)
                            nc.tensor.transpose(
                                out=tps[:],
                                in_=G_sb[:, bb * 64 : (bb + 1) * 64],
                                identity=ident_sb[:],
                            )
                            nc.scalar.activation(
                                out=hT[:, bb * 128 : (bb + 1) * 128],
                                in_=tps[:],
                                func=AF.Copy,
                            )
                if L == 2:
                    pps = pp.tile([NG, D], f32)
                    for bb in range(NB):
                        oh = sb.tile([128, NG], f32, bufs=2)
                        nc.vector.tensor_tensor(
                            out=oh[:],
                            in0=bat_sb[:, bb : bb + 1].to_broadcast([128, NG]),
                            in1=gid_sb[:],
                            op=ALU.is_equal,
                        )
                        nc.tensor.matmul(
                            out=pps[:],
                            lhsT=oh[:],
                            rhs=G_sb[:, bb * 64 : (bb + 1) * 64],
                            start=(bb == 0),
                            stop=(bb == NB - 1),
                        )
                    pool_sb = sb.tile([NG, D], f32)
                    nc.scalar.activation(out=pool_sb[:], in_=pps[:], func=AF.Copy)
                    nc.sync.dma_start(out=pooled_h[:], in_=pool_sb[:])
    if not nc.is_finalized():
        nc.finalize()
    return nc


LAST_RESULTS = None


def kernel(**inputs):
    from concourse.bass_utils import run_bass_kernel_spmd

    x = np.asarray(inputs["x"], np.float32)
    edge_index = np.asarray(inputs["edge_index"])
    batch = np.asarray(inputs["batch"])
    W = [np.asarray(inputs[k], np.float32) for k in ("W1", "W2", "W3")]
    b = [np.asarray(inputs[k], np.float32) for k in ("b1", "b2", "b3")]
    lin_w = np.asarray(inputs["lin_w"], np.float32)
    lin_b = np.asarray(inputs["lin_b"], np.float32)

    dis, bb_of, p_of, grp_cols, st_base, n_st_cols, gidx, dstl = _preprocess(
        edge_index
    )
    NSLOT = int(st_base[-1])
    NLC = dstl.shape[2]

    xT = np.zeros((NC, D, NPAD), np.float32)
    disc = np.zeros((NC, 128, NB), np.float32)
    batc = np.full((NC, 128, NB), -1.0, np.float32)
    for c in range(NC):
        nodes = np.arange(c * NPC, (c + 1) * NPC)
        col = bb_of[nodes] * 128 + p_of[nodes]
        xT[c][:, col] = x[nodes].T
        disc[c][p_of[nodes], bb_of[nodes]] = dis[nodes]
        batc[c][p_of[nodes], bb_of[nodes]] = batch[nodes].astype(np.float32)

    iota = np.ascontiguousarray(np.tile(np.arange(128, dtype=np.float32), (128, 1)))
    ident = np.eye(128, dtype=np.float32)
    gid = np.ascontiguousarray(np.tile(np.arange(NG, dtype=np.float32), (128, 1)))
    br = [
        np.ascontiguousarray(np.tile(bi.reshape(1, 1, D), (128, NB, 1)).reshape(128, NB * 64))
        for bi in b
    ]

    nc = _build_program(grp_cols, st_base, n_st_cols, NSLOT, NLC)
    in_maps = []
    for c in range(NC):
        in_maps.append(
            {
                "xT": np.ascontiguousarray(xT[c]),
                "disc": np.ascontiguousarray(disc[c]),
                "batc": np.ascontiguousarray(batc[c]),
                "gidx": np.ascontiguousarray(gidx[c]),
                "dstl": np.ascontiguousarray(dstl[c]),
                "w0": W[0],
                "w1": W[1],
                "w2": W[2],
                "br0": br[0],
                "br1": br[1],
                "br2": br[2],
                "iota": iota,
                "ident": ident,
                "gid": gid,
            }
        )
    import os

    trace = os.environ.get("KERNEL_TRACE", "") == "1"
    res = run_bass_kernel_spmd(nc, in_maps, list(range(NC)), trace=trace)
    global LAST_RESULTS
    LAST_RESULTS = res
    pooled = np.zeros((NG, D), np.float64)
    for r in res.results:
        pooled += r["pooled"].astype(np.float64)
    out = pooled.astype(np.float32) @ lin_w + lin_b
    return out.astype(np.float32)


# revision 4
# speedup vs baseline: 1.0433x; 1.0233x over previous
import numpy as np

N = 100000
D = 64
NG = 64
NC = 8
NPC = N // NC          # 12500 real nodes per core
NB = 98                # blocks of 128 dst nodes per core
NBA = 49               # blocks in each table half
NPAD = NB * 128        # 12544 padded nodes per core
HROWS = NBA * 128      # 6272 rows per core per half-table
NCH = 4                # src chunks (2 cores each)
WROWS = 2 * HROWS      # 12544 rows per (chunk, half) gather window
GSUB = 1024            # max idxs per dma_gather instruction
RG = [[0, 1, 2, 3, 4, 5, 6, 7]]
NST = NCH * 2          # gather streams: (chunk, src-half)
NGRP = NST * NB


def _preprocess(edge_index):
    src = edge_index[0].astype(np.int64)
    dst = edge_index[1].astype(np.int64)
    deg = np.bincount(dst, minlength=N) + 1  # +1 self loop
    dis = (1.0 / np.sqrt(deg.astype(np.float64))).astype(np.float32)

    core_of = np.arange(N) // NPC
    bb_of = np.empty(N, np.int64)
    p_of = np.empty(N, np.int64)
    for c in range(NC):
        nodes = np.arange(c * NPC, (c + 1) * NPC)
        order = np.argsort(-deg[nodes], kind="stable")
        r = np.empty(NPC, np.int64)
        r[order] = np.arange(NPC)
        bb_of[nodes] = r % NB
        p_of[nodes] = r // NB

    # half-table row: half = bb>=NBA; row(core,half) = (core%2)*HROWS + p*NBA + bb_h
    half_of = (bb_of >= NBA).astype(np.int64)
    bb_h = bb_of - half_of * NBA
    wrow_of = (core_of % 2) * HROWS + p_of * NBA + bb_h

    edata = []
    cnts = np.zeros((NC, NGRP), np.int64)
    dst_core = core_of[dst]
    for c in range(NC):
        m = dst_core == c
        es, ed = src[m], dst[m]
        st = (core_of[es] // 2) * 2 + half_of[es]     # stream = (chunk, half)
        gkey = st * NB + bb_of[ed]
        wrow = wrow_of[es]
        o = np.lexsort((wrow, gkey))
        gkey, wrow, pd = gkey[o], wrow[o], p_of[ed][o]
        cnts[c] = np.bincount(gkey, minlength=NGRP)
        edata.append((gkey, wrow, pd))

    gsz = np.maximum(cnts.max(axis=0), 16)

    grp_off = np.zeros(NGRP, np.int64)
    st_len = np.zeros(NST, np.int64)
    for s in range(NST):
        off = 0
        for bb in range(NB):
            g = s * NB + bb
            grp_off[g] = off
            off += gsz[g]
        st_len[s] = off
    st_len128 = ((st_len + 127) // 128) * 128
    st_base = np.zeros(NST + 1, np.int64)
    st_base[1:] = np.cumsum(st_len128)
    NSLOT = int(st_base[-1])
    n_st_cols = (st_len128 // 128).astype(np.int64)

    grp_cols = []
    lc_count = 0
    for s in range(NST):
        for bb in range(NB):
            g = s * NB + bb
            s0, s1 = grp_off[g], grp_off[g] + gsz[g]
            c0, c1 = s0 // 128, (s1 - 1) // 128
            cols = []
            for cc in range(c0, c1 + 1):
                cols.append((int(st_base[s]) // 128 + cc, lc_count))
                lc_count += 1
            grp_cols.append(cols)
    NLC = lc_count

    gidx = np.zeros((NC, 128, NSLOT // 16), np.int16)
    dstl = np.empty((NC, 128, NLC), np.float32)
    for c in range(NC):
        gkey, wrow, pd = edata[c]
        gi = np.zeros(NSLOT, np.int16)
        lab = np.full((NSLOT,), -1.0, np.float32)
        gstart = np.zeros(NGRP + 1, np.int64)
        gstart[1:] = np.cumsum(cnts[c])
        for g in range(NGRP):
            a, b = gstart[g], gstart[g + 1]
            base = int(st_base[g // NB] + grp_off[g])
            n = b - a
            gi[base : base + n] = wrow[a:b].astype(np.int16)
            lab[base : base + n] = pd[a:b].astype(np.float32)
        gidx[c] = np.tile(gi.reshape(-1, 16).T, (8, 1))
        dl = np.empty((128, NLC), np.float32)
        for g in range(NGRP):
            s0 = int(st_base[g // NB] + grp_off[g])
            s1 = s0 + int(gsz[g])
            for (tcol, lc) in grp_cols[g]:
                colbase = tcol * 128
                col = np.full(128, -1.0, np.float32)
                lo, hi = max(s0, colbase), min(s1, colbase + 128)
                col[lo - colbase : hi - colbase] = lab[lo:hi]
                dl[:, lc] = col
        dstl[c] = dl
    return dis, bb_of, p_of, grp_cols, st_base, n_st_cols, gidx, dstl


def _build_program(grp_cols, st_base, n_st_cols, NSLOT, NLC):
    from concourse import bacc, mybir

    import concourse.tile as tile

    f32 = mybir.dt.float32
    i16 = mybir.dt.int16
    AF = mybir.ActivationFunctionType
    ALU = mybir.AluOpType

    nc = bacc.Bacc(None, target_bir_lowering=False)
    xT_h = nc.declare_dram_parameter("xT", [D, NPAD], f32, False)
    disc_h = nc.declare_dram_parameter("disc", [128, NB], f32, False)
    batc_h = nc.declare_dram_parameter("batc", [128, NB], f32, False)
    gidx_h = nc.declare_dram_parameter("gidx", [128, NSLOT // 16], i16, False)
    dstl_h = nc.declare_dram_parameter("dstl", [128, NLC], f32, False)
    w_h = [nc.declare_dram_parameter(f"w{i}", [D, D], f32, False) for i in range(3)]
    br_h = [
        nc.declare_dram_parameter(f"br{i}", [128, NB * 64], f32, False)
        for i in range(3)
    ]
    iota_h = nc.declare_dram_parameter("iota", [128, 128], f32, False)
    ident_h = nc.declare_dram_parameter("ident", [128, 128], f32, False)
    gid_h = nc.declare_dram_parameter("gid", [128, NG], f32, False)
    pooled_h = nc.declare_dram_parameter("pooled", [NG, D], f32, True)

    g_loc = [
        [
            nc.dram_tensor(f"g_loc{L}_{h}", [128, NBA * 64], f32, kind="Internal")
            for h in range(2)
        ]
        for L in range(3)
    ]
    g_ful = [
        [
            nc.dram_tensor(
                f"g_ful{L}_{h}",
                [NC * HROWS, 64],
                f32,
                kind="Internal",
                addr_space="Shared",
            )
            for h in range(2)
        ]
        for L in range(3)
    ]

    from concourse import library_config

    with tile.TileContext(nc) as tc:
        with tc.tile_pool(name="sb", bufs=1) as sb, tc.tile_pool(
            name="pp", bufs=1, space="PSUM"
        ) as pp:
            nc.gpsimd.load_library(library_config.mlp)
            hT = sb.tile([D, NPAD], f32)
            nc.sync.dma_start(out=hT[:], in_=xT_h[:])
            dis_sb = sb.tile([128, NB], f32)
            nc.sync.dma_start(out=dis_sb[:], in_=disc_h[:])
            bat_sb = sb.tile([128, NB], f32)
            nc.sync.dma_start(out=bat_sb[:], in_=batc_h[:])
            gidx_sb = sb.tile([128, NSLOT // 16], i16)
            nc.sync.dma_start(out=gidx_sb[:], in_=gidx_h[:])
            dstl_sb = sb.tile([128, NLC], f32)
            nc.sync.dma_start(out=dstl_sb[:], in_=dstl_h[:])
            w_sb = []
            for i in range(3):
                wt = sb.tile([D, D], f32, name=f"w_sb{i}")
                nc.sync.dma_start(out=wt[:], in_=w_h[i][:])
                w_sb.append(wt)
            br_sb = sb.tile([128, NB * 64], f32)
            iota_sb = sb.tile([128, 128], f32)
            nc.sync.dma_start(out=iota_sb[:], in_=iota_h[:])
            ident_sb = sb.tile([128, 128], f32)
            nc.sync.dma_start(out=ident_sb[:], in_=ident_h[:])
            gid_sb = sb.tile([128, NG], f32)
            nc.sync.dma_start(out=gid_sb[:], in_=gid_h[:])

            G_sb = sb.tile([128, NB * 64], f32)
            A_sb = sb.tile([128, NB * 64], f32)
            reg1024 = nc.gpsimd.to_reg(GSUB)
            tail_regs = {GSUB: reg1024}
            HS = [slice(0, NBA * 64), slice(NBA * 64, NB * 64)]

            for L in range(3):
                # transform + per-half table publish
                for h in range(2):
                    for bb in range(h * NBA, (h + 1) * NBA):
                        gps = pp.tile([128, D], f32, bufs=2)
                        nc.tensor.matmul(
                            out=gps[:],
                            lhsT=hT[:, bb * 128 : (bb + 1) * 128],
                            rhs=w_sb[L][:],
                            start=True,
                            stop=True,
                        )
                        nc.vector.tensor_tensor(
                            out=G_sb[:, bb * 64 : (bb + 1) * 64],
                            in0=gps[:],
                            in1=dis_sb[:, bb : bb + 1].to_broadcast([128, 64]),
                            op=ALU.mult,
                        )
                    nc.sync.dma_start(out=g_loc[L][h][:], in_=G_sb[:, HS[h]])
                    nc.gpsimd.collective_compute(
                        "AllGather",
                        ALU.bypass,
                        replica_groups=RG,
                        ins=[g_loc[L][h][:]],
                        outs=[g_ful[L][h][:]],
                    )

                # gathers: per (chunk, half) stream, 1024-idx windows
                msgs_of_col = {}
                for s in range(NST):
                    ch, h = s // 2, s % 2
                    win = g_ful[L][h][ch * WROWS : (ch + 1) * WROWS, :]
                    nslots = int(n_st_cols[s]) * 128
                    base = int(st_base[s])
                    for k in range(0, nslots, GSUB):
                        cnt = min(GSUB, nslots - k)
                        if cnt not in tail_regs:
                            tail_regs[cnt] = nc.gpsimd.to_reg(cnt)
                        mt = sb.tile([128, GSUB // 128, 64], f32, name="msgs", bufs=8)
                        nc.gpsimd.dma_gather(
                            out_ap=mt[:, : cnt // 128, :],
                            in_ap=win,
                            idxs_ap=gidx_sb[
                                :, (base + k) // 16 : (base + k + cnt) // 16
                            ],
                            num_idxs=cnt,
                            num_idxs_reg=tail_regs[cnt],
                            elem_size=64,
                        )
                        for j in range(cnt // 128):
                            msgs_of_col[(base + k) // 128 + j] = (mt, j)

                # scatter
                for g in range(NGRP):
                    s, bb = g // NB, g % NB
                    cols = grp_cols[g]
                    ncols = len(cols)
                    sel = sb.tile([128, 8 * 128], f32, name="sel", bufs=6)
                    for ci, (tcol, lc) in enumerate(cols):
                        nc.vector.tensor_tensor(
                            out=sel[:, ci * 128 : (ci + 1) * 128],
                            in0=dstl_sb[:, lc : lc + 1].to_broadcast([128, 128]),
                            in1=iota_sb[:],
                            op=ALU.is_equal,
                        )
                    aps = pp.tile([128, D], f32, bufs=3)
                    for ci, (tcol, lc) in enumerate(cols):
                        mt, j = msgs_of_col[tcol]
                        nc.tensor.matmul(
                            out=aps[:],
                            lhsT=sel[:, ci * 128 : (ci + 1) * 128],
                            rhs=mt[:, j, :],
                            start=(ci == 0),
                            stop=(ci == ncols - 1),
                        )
                    bsl = slice(bb * 64, (bb + 1) * 64)
                    if s == 0:
                        nc.scalar.activation(
                            out=A_sb[:, bsl], in_=aps[:], func=AF.Copy
                        )
                    else:
                        nc.vector.tensor_add(
                            out=A_sb[:, bsl], in0=A_sb[:, bsl], in1=aps[:]
                        )

                # epilogue per dst half: self-loop, scale, bias, relu
                nc.sync.dma_start(out=br_sb[:], in_=br_h[L][:])
                for h in range(2):
                    hs = HS[h]
                    dsl = dis_sb[:, h * NBA : (h + 1) * NBA]
                    nc.vector.tensor_add(
                        out=A_sb[:, hs], in0=A_sb[:, hs], in1=G_sb[:, hs]
                    )
                    nc.vector.tensor_tensor(
                        out=A_sb[:, hs],
                        in0=A_sb[:, hs],
                        in1=dsl.to_broadcast([128, NBA, 64]),
                        op=ALU.mult,
                    )
                    nc.vector.tensor_add(
                        out=G_sb[:, hs], in0=A_sb[:, hs], in1=br_sb[:, hs]
                    )
                    nc.scalar.activation(
                        out=G_sb[:, hs], in_=G_sb[:, hs], func=AF.Relu
                    )
                    if L < 2:
                        for bb in range(h * NBA, (h + 1) * NBA):
                            tps = pp.tile([D, 128], f32, bufs=2)
                            nc.tensor.transpose(
                                out=tps[:],
                                in_=G_sb[:, bb * 64 : (bb + 1) * 64],
                                identity=ident_sb[:],
                            )
                            nc.scalar.activation(
                                out=hT[:, bb * 128 : (bb + 1) * 128],
                                in_=tps[:],
                                func=AF.Copy,
                            )
                if L == 2:
                    pps = pp.tile([NG, D], f32)
                    for bb in range(NB):
                        oh = sb.tile([128, NG], f32, bufs=2)
                        nc.vector.tensor_tensor(
                            out=oh[:],
                            in0=bat_sb[:, bb : bb + 1].to_broadcast([128, NG]),
                            in1=gid_sb[:],
                            op=ALU.is_equal,
                        )
                        nc.tensor.matmul(
                            out=pps[:],
                            lhsT=oh[:],
                            rhs=G_sb[:, bb * 64 : (bb + 1) * 64],
                            start=(bb == 0),
                            stop=(bb == NB - 1),
                        )
                    pool_sb = sb.tile([NG, D], f32)
                    nc.scalar.activation(out=pool_sb[:], in_=pps[:], func=AF.Copy)
                    nc.sync.dma_start(out=pooled_h[:], in_=pool_sb[:])
    if not nc.is_finalized():
        nc.finalize()
    return nc


LAST_RESULTS = None


def kernel(**inputs):
    from concourse.bass_utils import run_bass_kernel_spmd

    x = np.asarray(inputs["x"], np.float32)
    edge_index = np.asarray(inputs["edge_index"])
    batch = np.asarray(inputs["batch"])
    W = [np.asarray(inputs[k], np.float32) for k in ("W1", "W2", "W3")]
    b = [np.asarray(inputs[k], np.float32) for k in ("b1", "b2", "b3")]
    lin_w = np.asarray(inputs["lin_w"], np.float32)
    lin_b = np.asarray(inputs["lin_b"], np.float32)

    dis, bb_of, p_of, grp_cols, st_base, n_st_cols, gidx, dstl = _preprocess(
        edge_index
    )
    NSLOT = int(st_base[-1])
    NLC = dstl.shape[2]

    xT = np.zeros((NC, D, NPAD), np.float32)
    disc = np.zeros((NC, 128, NB), np.float32)
    batc = np.full((NC, 128, NB), -1.0, np.float32)
    for c in range(NC):
        nodes = np.arange(c * NPC, (c + 1) * NPC)
        col = bb_of[nodes] * 128 + p_of[nodes]
        xT[c][:, col] = x[nodes].T
        disc[c][p_of[nodes], bb_of[nodes]] = dis[nodes]
        batc[c][p_of[nodes], bb_of[nodes]] = batch[nodes].astype(np.float32)

    iota = np.ascontiguousarray(np.tile(np.arange(128, dtype=np.float32), (128, 1)))
    ident = np.eye(128, dtype=np.float32)
    gid = np.ascontiguousarray(np.tile(np.arange(NG, dtype=np.float32), (128, 1)))
    br = [
        np.ascontiguousarray(np.tile(bi.reshape(1, 1, D), (128, NB, 1)).reshape(128, NB * 64))
        for bi in b
    ]

    nc = _build_program(grp_cols, st_base, n_st_cols, NSLOT, NLC)
    in_maps = []
    for c in range(NC):
        in_maps.append(
            {
                "xT": np.ascontiguousarray(xT[c]),
                "disc": np.ascontiguousarray(disc[c]),
                "batc": np.ascontiguousarray(batc[c]),
                "gidx": np.ascontiguousarray(gidx[c]),
                "dstl": np.ascontiguousarray(dstl[c]),
                "w0": W[0],
                "w1": W[1],
                "w2": W[2],
                "br0": br[0],
                "br1": br[1],
                "br2": br[2],
                "iota": iota,
                "ident": ident,
                "gid": gid,
            }
        )
    import os

    trace = os.environ.get("KERNEL_TRACE", "") == "1"
    res = run_bass_kernel_spmd(nc, in_maps, list(range(NC)), trace=trace)
    global LAST_RESULTS
    LAST_RESULTS = res
    pooled = np.zeros((NG, D), np.float64)
    for r in res.results:
        pooled += r["pooled"].astype(np.float64)
    out = pooled.astype(np.float32) @ lin_w + lin_b
    return out.astype(np.float32)
